# revision 1
# baseline (speedup 1.0000x reference)
"""Trainium2 Bass kernel for BilinearClassification (segment_reduce).

Math (per example b):
  ent[e,:]  = masked-mean over subword span of hidden[idx[e,s],:]      (E=64, H=768)
  subj[t,:] = ent[trip[t,0],:] * pm[t];  obj[t,:] = ent[trip[t,1],:] * pm[t]
  bl[t, (g,i,j)] = subj[t, g*8+i] * obj[t, g*8+j]                      (f = 6144)
  logits[t,n] = bl[t,:] @ W[:,n] + b[n]                                (NT=42)

Device strategy (8 cores, 4 examples each, no collectives, all-bf16 inputs):
  - host precomputes one-hot segment-mean matrix AT[l, (ex,e)] (mask, 1/cnt and
    example-pair block-diagonal folded) and pair-stacked triplet one-hots
    G_s/G_o [128=(2ex x 64e), 256=(2ex x 128t)] (pair_mask folded), so the
    device only does matmuls + copies + one elementwise mul.
  - stage 1: ent = AT.T @ hidden; two 384-col PSUM halves per example pair.
  - tables: ent_exp_s[(ex,e), (g,i,j)] = ent[(ex,e), 8g+i] (and _o with 8g+j),
    built by broadcast-AP copies (ACT/DVE for the first slices, idle GPSIMD
    for the rest) because matmul weights APs must be 2D.
  - stage 2 per f-chunk c (128 f-rows = 2 bilinear groups x 8i x 8j):
      S_exp = ent_exp_s[:, c-slice].T @ G_s   -> PSUM [128, 256]
      O_exp = ent_exp_o[:, c-slice].T @ G_o   -> PSUM [128, 256]
      s_sb  = ACT evac of S_exp; blT_c = s_sb * O_exp (DVE) -> SBUF bf16
      logits += W_c.T @ blT_c                 (PSUM accumulate over 48 chunks)
  - host adds b_fc and reshapes.
"""
import sys

sys.path.insert(0, "/opt/trn_rl_repo")

import numpy as np

import concourse.bass as bass
import concourse.bacc as bacc
import concourse.tile as tile
from concourse import mybir
from concourse.bass_utils import run_bass_kernel_spmd

F32 = mybir.dt.float32
BF16 = mybir.dt.bfloat16

B, L, H = 32, 512, 768
E, S, T = 64, 8, 128
NT = 42
NCORES = 8
EXPC = B // NCORES          # 4 examples per core
NPAIR = EXPC // 2           # 2 example-pairs per core
KC = L // 128               # 4 contraction chunks over l
FC = (H * 8) // 128         # 48 f-chunks
TP = 2 * T                  # 256 columns per pair (2ex x 128t)
NH = 2                      # ent psum halves (384 cols each)
HHALF = H // NH


def build_program(reps=1):
    """reps>1 repeats the whole body back-to-back (for wall-clock timing
    amplification in the test harness; the grading path uses reps=1)."""
    nc = bacc.Bacc("TRN2", target_bir_lowering=False, debug=False)

    hid_d = nc.dram_tensor("hid", (EXPC * L, H), BF16, kind="ExternalInput")
    # AT is pair-block-diagonal: rows (ex,kc,l), cols (ex' * 64 + e)
    at_d = nc.dram_tensor("at", (EXPC * L, 2 * E), BF16, kind="ExternalInput")
    gs_d = nc.dram_tensor("gs", (NPAIR, 128, TP), BF16, kind="ExternalInput")
    go_d = nc.dram_tensor("go", (NPAIR, 128, TP), BF16, kind="ExternalInput")
    # W pre-shuffled on host to the SBUF chunk layout [p, (c n)]
    w_d = nc.dram_tensor("w", (128, FC * NT), BF16, kind="ExternalInput")
    out_d = nc.dram_tensor("out", (NPAIR, NT, TP), F32, kind="ExternalOutput")

    with tile.TileContext(nc) as tc:
        with (
            tc.tile_pool(name="consts", bufs=1) as consts,
            tc.tile_pool(name="hidp", bufs=4) as hidp,
            tc.tile_pool(name="atp", bufs=2) as atp,
            tc.tile_pool(name="entps", bufs=2, space="PSUM") as entps,
            tc.tile_pool(name="entsb", bufs=2) as entsb,
            tc.tile_pool(name="tabp", bufs=2) as tabp,
            tc.tile_pool(name="sops", bufs=2, space="PSUM") as sops,
            tc.tile_pool(name="lgps", bufs=2, space="PSUM") as lgps,
            tc.tile_pool(name="blp", bufs=6) as blp,
            tc.tile_pool(name="outp", bufs=2) as outp,
        ):
          for _rep in range(reps):
            # ---- input DMAs, consolidated (HWDGE setup is ~600ns per DMA)
            # and ordered so pair 0's ent inputs land first
            hid_t = [None] * NPAIR   # [128, 8ck, H] per pair
            at_t = [None] * NPAIR    # [128, 8ck, 2E] per pair
            gs_t = [None] * NPAIR
            go_t = [None] * NPAIR
            w_all = None
            for P in range(NPAIR):
                att = atp.tile([128, 2 * KC, 2 * E], BF16)
                at_r = at_d[:].rearrange("(ck p) e -> p ck e", p=128)
                nc.sync.dma_start(att[:], at_r[:, P * 2 * KC : (P + 1) * 2 * KC, :])
                at_t[P] = att
                hid_r = hid_d[:].rearrange("(ck p) h -> p ck h", p=128)
                hts = []
                for half in range(2):  # separate tiles so deps are per-half
                    h1 = hidp.tile([128, KC, H], BF16)
                    if P == 0 and half == 0:
                        # finest granularity on the critical first chunks so
                        # the very first ent matmul starts as early as possible
                        for ck in range(KC):
                            nc.sync.dma_start(
                                h1[:, ck, :], hid_r[:, P * 2 * KC + ck, :])
                    else:
                        nc.sync.dma_start(
                            h1[:],
                            hid_r[:, P * 2 * KC + half * KC : P * 2 * KC + (half + 1) * KC, :])
                    hts.append(h1)
                hid_t[P] = hts
                g1 = consts.tile([128, TP], BF16, tag=f"gs{P}")
                nc.sync.dma_start(g1[:], gs_d[P])
                gs_t[P] = g1
                g2 = consts.tile([128, TP], BF16, tag=f"go{P}")
                nc.sync.dma_start(g2[:], go_d[P])
                go_t[P] = g2
                if P == 0:
                    w_all = consts.tile([128, FC, NT], BF16, tag="w")
                    nc.sync.dma_start(
                        w_all[:], w_d[:].rearrange("p (c n) -> p c n", n=NT))

            # ---- stage 1 + tables for every pair first (program order)
            tabs = []
            for P in range(NPAIR):
                ent_sb = entsb.tile([128, H], BF16, tag="ent_sb")
                tab_s = tabp.tile([128, H * 8], BF16, tag="tab_s")
                tab_o = tabp.tile([128, H * 8], BF16, tag="tab_o")
                for nh in range(NH):
                    fast = P == 0 and nh == 0
                    ent_ps = entps.tile([128, HHALF], F32)
                    for ck in range(2 * KC):
                        nc.tensor.matmul(
                            ent_ps[:],
                            at_t[P][:, ck, :],
                            hid_t[P][ck // KC][:, ck % KC,
                                               nh * HHALF : (nh + 1) * HHALF],
                            start=(ck == 0),
                            stop=(ck == 2 * KC - 1),
                        )
                    if not fast:
                        # GPSIMD has no PSUM port: stage this half into SBUF
                        nc.scalar.copy(
                            ent_sb[:, nh * HHALF : (nh + 1) * HHALF], ent_ps[:])
                    # table slices covering this half: groups [nh*48, (nh+1)*48)
                    g0 = nh * (96 // NH)
                    gn2 = 96 // NH // 2   # two slices per half
                    for sl in range(2):
                        ga = g0 + sl * gn2
                        if fast:
                            # straight from PSUM on the idle fast engines so
                            # pair 0 stage 2 starts as early as possible
                            src = ent_ps[:, (ga - g0) * 8 : (ga - g0 + gn2) * 8]
                        else:
                            src = ent_sb[:, ga * 8 : (ga + gn2) * 8]
                        src_s = (src.rearrange("p (g i) -> p g i", i=8)
                                 .unsqueeze(3).broadcast_to((128, gn2, 8, 8)))
                        src_o = (src.rearrange("p (g j) -> p g j", j=8)
                                 .unsqueeze(2).broadcast_to((128, gn2, 8, 8)))
                        dst_s = tab_s[:, ga * 64 : (ga + gn2) * 64].rearrange(
                            "p (g i j) -> p g i j", i=8, j=8)
                        dst_o = tab_o[:, ga * 64 : (ga + gn2) * 64].rearrange(
                            "p (g i j) -> p g i j", i=8, j=8)
                        if fast:
                            nc.scalar.copy(dst_s, src_s)
                            nc.vector.tensor_copy(dst_o, src_o)
                        else:
                            # on GPSIMD, overlapped with running stage 2
                            nc.gpsimd.tensor_copy(dst_s, src_s)
                            nc.gpsimd.tensor_copy(dst_o, src_o)
                tabs.append((tab_s, tab_o))

            # ---- stage 2: 48 f-chunks per pair, merged in pairs of chunks
            for P in range(NPAIR):
                tab_s, tab_o = tabs[P]
                lg_ps = lgps.tile([NT, TP], F32)
                for cc in range(FC // 2):
                    s_ps = sops.tile([128, 2, TP], F32, tag="s")
                    o_ps = sops.tile([128, 2, TP], F32, tag="o")
                    for h in range(2):
                        c = cc * 2 + h
                        nc.tensor.matmul(
                            s_ps[:, h, :],
                            tab_s[:, c * 128 : (c + 1) * 128],
                            gs_t[P][:],
                            start=True,
                            stop=True,
                        )
                        nc.tensor.matmul(
                            o_ps[:, h, :],
                            tab_o[:, c * 128 : (c + 1) * 128],
                            go_t[P][:],
                            start=True,
                            stop=True,
                        )
                    # DVE can read at most one PSUM operand: evacuate S via ACT
                    s_sb = blp.tile([128, 2, TP], F32, tag="s_sb")
                    nc.scalar.copy(s_sb[:], s_ps[:])
                    blt = blp.tile([128, 2, TP], BF16, tag="blt")
                    nc.vector.tensor_mul(blt[:], s_sb[:], o_ps[:])
                    for h in range(2):
                        c = cc * 2 + h
                        nc.tensor.matmul(
                            lg_ps[:],
                            w_all[:, c, :],
                            blt[:, h, :],
                            start=(c == 0),
                            stop=(c == FC - 1),
                        )

                out_sb = outp.tile([NT, TP], F32)
                nc.scalar.copy(out_sb[:], lg_ps[:])
                nc.sync.dma_start(out_d[P], out_sb[:])

    nc.compile()
    return nc


def host_prep(hidden_states, entity_subw_indices, entity_subw_mask,
              triplet_entity_nums, pair_mask, W_fc):
    """Build per-core input maps (numpy only, cheap)."""
    import ml_dtypes
    bf16 = ml_dtypes.bfloat16
    hs = np.asarray(hidden_states, dtype=np.float32).astype(bf16)
    idx = np.asarray(entity_subw_indices)
    msk = np.asarray(entity_subw_mask).astype(np.float32)
    trip = np.asarray(triplet_entity_nums)
    pm = np.asarray(pair_mask).astype(np.float32)
    # shuffle W to the SBUF chunk layout [p, (c, n)]
    w = (np.asarray(W_fc, dtype=np.float32).reshape(FC, 128, NT)
         .transpose(1, 0, 2).reshape(128, FC * NT).astype(bf16))

    # AT[b]: (L, 2E) pair-block-diagonal with mask/cnt folded
    cnt = np.maximum(msk.sum(axis=2), 1.0)          # (B, E)
    wgt = msk / cnt[:, :, None]                     # (B, E, S)
    at = np.zeros((B, L, 2 * E), np.float32)
    b_i, e_i, s_i = np.nonzero(msk > 0)
    np.add.at(at, (b_i, idx[b_i, e_i, s_i], (b_i % 2) * E + e_i),
              wgt[b_i, e_i, s_i])
    at = at.astype(bf16)

    # pair-stacked block-diagonal triplet one-hots (2ex x 64e, 2ex x 128t)
    gs = np.zeros((B // 2, 128, TP), bf16)
    go = np.zeros((B // 2, 128, TP), bf16)
    bb = np.arange(B)[:, None]
    tt = np.arange(T)[None, :]
    pair = bb // 2
    exl = (bb % 2)
    gs[pair, exl * E + trip[:, :, 0], exl * T + tt] = pm.astype(bf16)
    go[pair, exl * E + trip[:, :, 1], exl * T + tt] = pm.astype(bf16)

    in_maps = []
    for c in range(NCORES):
        b0 = c * EXPC
        in_maps.append({
            "hid": np.ascontiguousarray(hs[b0 : b0 + EXPC].reshape(EXPC * L, H)),
            "at": np.ascontiguousarray(
                at[b0 : b0 + EXPC].reshape(EXPC * L, 2 * E)),
            "gs": np.ascontiguousarray(gs[b0 // 2 : b0 // 2 + NPAIR]),
            "go": np.ascontiguousarray(go[b0 // 2 : b0 // 2 + NPAIR]),
            "w": w,
        })
    return in_maps


def assemble(results, b_fc):
    """results[c]["out"] is (NPAIR, NT, 2ex x 128t) -> (B, T, NT) + bias."""
    logits = np.empty((B, T, NT), np.float32)
    for c in range(NCORES):
        o = results[c]["out"].reshape(NPAIR, NT, 2, T)
        for P in range(NPAIR):
            for exl in range(2):
                b = c * EXPC + P * 2 + exl
                logits[b] = o[P, :, exl, :].T
    return logits + np.asarray(b_fc, np.float32)[None, None, :]


_NC_CACHE = None


def kernel(hidden_states, entity_subw_indices, entity_subw_mask,
           triplet_entity_nums, pair_mask, W_fc, b_fc):
    global _NC_CACHE
    if _NC_CACHE is None:
        _NC_CACHE = build_program()
    nc = _NC_CACHE
    in_maps = host_prep(hidden_states, entity_subw_indices, entity_subw_mask,
                        triplet_entity_nums, pair_mask, W_fc)
    res = run_bass_kernel_spmd(nc, in_maps, core_ids=list(range(NCORES)))
    return assemble(res.results, b_fc)



# revision 5
# speedup vs baseline: 2.6246x; 2.6246x over previous
"""Trainium2 Bass kernel for BilinearClassification (segment_reduce).

Math (per example b):
  ent[e,:]  = masked-mean over subword span of hidden[idx[e,s],:]      (E=64, H=768)
  subj[t,:] = ent[trip[t,0],:] * pm[t];  obj[t,:] = ent[trip[t,1],:] * pm[t]
  bl[t, (g,i,j)] = subj[t, g*8+i] * obj[t, g*8+j]                      (f = 6144)
  logits[t,n] = bl[t,:] @ W[:,n] + b[n]                                (NT=42)

Device strategy (8 cores, 4 examples = 2 example-pairs each, no collectives):
  - Only VALID triplets (pair_mask) are computed, packed into C=128 columns
    per example-pair (host pairs examples to balance; the few overflow
    triplets are computed exactly on the host). Masked slots get b_fc.
  - Host gathers only the DISTINCT hidden rows each example references
    (K*128 rows instead of L=512), shrinking hid DMA and stage-1 PE.
  - stage 1: ent[(ex,e), h] = AT.T @ hidg per pair (2K contraction chunks).
  - stage 2 per pair:
      pre-gather:  sG[h, t] = ent_view.T @ G_s  (6 matmuls), evac to SBUF;
                   oG likewise.
      per f-chunk c (128 f-rows = 2 groups x 8i x 8j), via constant 0/1
      expansion patterns P (lhsT must be 2D, so partition-expansion is a
      matmul):  S_c = P_s[c%8].T @ sG[hc]   -> PSUM
                O_c = P_o[c%8].T @ oG[hc]   -> PSUM
      s_sb = ACT evac of S group; blt = s_sb * O (DVE) -> SBUF bf16
      logits += W_c.T @ blt_c               (PSUM accumulate over 48 chunks)
  - host scatters packed columns back to (b, t) and adds b_fc.
"""
import sys

sys.path.insert(0, "/opt/trn_rl_repo")

import numpy as np

import concourse.bass as bass
import concourse.bacc as bacc
import concourse.tile as tile
from concourse import mybir
from concourse.bass_utils import run_bass_kernel_spmd

F32 = mybir.dt.float32
BF16 = mybir.dt.bfloat16

B, L, H = 32, 512, 768
E, S, T = 64, 8, 128
NT = 42
NCORES = 8
EXPC = B // NCORES          # 4 examples per core
NPAIR = EXPC // 2           # 2 example-pairs per core
FC = (H * 8) // 128         # 48 f-chunks
HC = H // 128               # 6 h-chunks

# data-dependent compile parameters (set by host_prep; defaults match the
# bundled fixed-seed inputs)
_LAST_C = 128               # packed triplet columns per pair
_LAST_K = 2                 # 128-row contraction chunks per example


def _groups(C):
    """(GS, GO): s/o chunk-group sizes. PSUM slots are C*4 bytes; tiles must
    not let a matmul output slot straddle a 2KB bank."""
    slot = 4 * C
    if 2048 % slot == 0:
        return 4, 8
    g = max(1, 2048 // slot)
    g = 4 if g >= 4 else g
    while FC % g:
        g -= 1
    return g, g


def build_program(reps=1, C=None, K=None):
    C = _LAST_C if C is None else C
    K = _LAST_K if K is None else K
    GS, GO = _groups(C)
    nc = bacc.Bacc("TRN2", target_bir_lowering=False, debug=False)

    KK = 2 * K              # contraction chunks per pair
    HH = H // 2
    hid_d = nc.dram_tensor("hid", (NPAIR * KK * 128, H), BF16, kind="ExternalInput")
    at_d = nc.dram_tensor("at", (NPAIR * KK * 128, 2 * E), BF16, kind="ExternalInput")
    gs_d = nc.dram_tensor("gs", (NPAIR, 128, C), BF16, kind="ExternalInput")
    go_d = nc.dram_tensor("go", (NPAIR, 128, C), BF16, kind="ExternalInput")
    pat_d = nc.dram_tensor("pat", (16 * 128, 128), BF16, kind="ExternalInput")
    # W pre-shuffled on host to the SBUF chunk layout [p, (c n)]
    w_d = nc.dram_tensor("w", (128, FC * NT), BF16, kind="ExternalInput")
    out_d = nc.dram_tensor("out", (NT, NPAIR, C), F32, kind="ExternalOutput")

    with tile.TileContext(nc) as tc:
        with (
            tc.tile_pool(name="consts", bufs=1) as consts,
            tc.tile_pool(name="hidp", bufs=2 * KK) as hidp,
            tc.tile_pool(name="atp", bufs=2) as atp,
            tc.tile_pool(name="entsb", bufs=1) as entsb,
            tc.tile_pool(name="sps", bufs=2, space="PSUM") as sps,
            tc.tile_pool(name="ops", bufs=2, space="PSUM") as ops,
            tc.tile_pool(name="lgps", bufs=1, space="PSUM") as lgps,
            tc.tile_pool(name="gsbp", bufs=2) as gsbp,
            tc.tile_pool(name="ssbp", bufs=2) as ssbp,
            tc.tile_pool(name="bltp", bufs=2) as bltp,
            tc.tile_pool(name="outp", bufs=1) as outp,
        ):
          for _rep in range(reps):
            # ---- input DMAs (pair 0's stage-1 inputs first)
            hid_t = [None] * NPAIR   # list of KK tiles [128, H] per pair
            at_t = [None] * NPAIR    # [128, KK, 2E]
            gs_t = [None] * NPAIR
            go_t = [None] * NPAIR
            hid_r = hid_d[:].rearrange("(pk p) h -> p pk h", p=128)
            at_r = at_d[:].rearrange("(pk p) e -> p pk e", p=128)
            for P in range(NPAIR):
                att = atp.tile([128, KK, 2 * E], BF16)
                nc.sync.dma_start(att[:], at_r[:, P * KK : (P + 1) * KK, :])
                at_t[P] = att
                hts = []
                for ck in range(KK):
                    h1 = hidp.tile([128, H], BF16)
                    nc.sync.dma_start(h1[:], hid_r[:, P * KK + ck, :])
                    hts.append(h1)
                hid_t[P] = hts
                g1 = consts.tile([128, C], BF16, tag=f"gs{P}")
                nc.sync.dma_start(g1[:], gs_d[P])
                gs_t[P] = g1
                g2 = consts.tile([128, C], BF16, tag=f"go{P}")
                nc.sync.dma_start(g2[:], go_d[P])
                go_t[P] = g2
            pat_t = consts.tile([128, 16, 128], BF16, tag="pat")
            nc.sync.dma_start(
                pat_t[:], pat_d[:].rearrange("(sb p) f -> p sb f", p=128))
            w_all = consts.tile([128, FC, NT], BF16, tag="w")
            nc.sync.dma_start(
                w_all[:], w_d[:].rearrange("p (c n) -> p c n", n=NT))

            # ---- stage 1 for both pairs first (ent PSUM borrows an s tile)
            ent_sb = []
            for P in range(NPAIR):
                esb = entsb.tile([128, H], BF16, tag=f"ent{P}")
                for nh in range(2):
                    ent_tile = sps.tile([128, GS, C], F32, tag="s")
                    ent_ps = ent_tile[:].rearrange("p a c -> p (a c)")[:, :HH]
                    for ck in range(KK):
                        nc.tensor.matmul(
                            ent_ps,
                            at_t[P][:, ck, :],
                            hid_t[P][ck][:, nh * HH : (nh + 1) * HH],
                            start=(ck == 0),
                            stop=(ck == KK - 1),
                        )
                    if nh == 0:
                        nc.scalar.copy(esb[:, :HH], ent_ps)
                    else:
                        nc.vector.tensor_copy(esb[:, HH:], ent_ps)
                ent_sb.append(esb)

            # ---- stage 2
            lg = lgps.tile([NT, NPAIR, C], F32)
            for P in range(NPAIR):
                esb = ent_sb[P]
                # pre-gather subj/obj at h-granularity: [128h, HC, C]
                sg_ps = ops.tile([128, GO, C], F32, tag="o")
                for hc in range(HC):
                    nc.tensor.matmul(
                        sg_ps[:, hc, :], esb[:, 128 * hc : 128 * (hc + 1)],
                        gs_t[P][:], start=True, stop=True)
                sg_sb = gsbp.tile([128, HC, C], BF16, tag="sg")
                nc.scalar.copy(sg_sb[:], sg_ps[:, :HC, :])
                og_ps = ops.tile([128, GO, C], F32, tag="o")
                for hc in range(HC):
                    nc.tensor.matmul(
                        og_ps[:, hc, :], esb[:, 128 * hc : 128 * (hc + 1)],
                        go_t[P][:], start=True, stop=True)
                og_sb = gsbp.tile([128, HC, C], BF16, tag="og")
                nc.vector.tensor_copy(og_sb[:], og_ps[:, :HC, :])

                for g in range(FC // GO):
                    o_ps = ops.tile([128, GO, C], F32, tag="o")
                    s_sb = ssbp.tile([128, GO, C], BF16)
                    for h2 in range(GO // GS):
                        s_ps = sps.tile([128, GS, C], F32, tag="s")
                        for j in range(GS):
                            c = g * GO + h2 * GS + j
                            nc.tensor.matmul(
                                s_ps[:, j, :], pat_t[:, c % 8, :],
                                sg_sb[:, c // 8, :], start=True, stop=True)
                        for j in range(GS):
                            c = g * GO + h2 * GS + j
                            nc.tensor.matmul(
                                o_ps[:, h2 * GS + j, :], pat_t[:, 8 + c % 8, :],
                                og_sb[:, c // 8, :], start=True, stop=True)
                        nc.scalar.copy(
                            s_sb[:, h2 * GS : (h2 + 1) * GS, :], s_ps[:])
                    blt = bltp.tile([128, GO, C], BF16)
                    nc.vector.tensor_mul(blt[:], s_sb[:], o_ps[:])
                    for j in range(GO):
                        c = g * GO + j
                        nc.tensor.matmul(
                            lg[:, P, :],
                            w_all[:, c, :],
                            blt[:, j, :],
                            start=(c == 0),
                            stop=(c == FC - 1),
                        )

            out_sb = outp.tile([NT, NPAIR, C], F32)
            nc.scalar.copy(out_sb[:], lg[:])
            nc.sync.dma_start(out_d[:], out_sb[:])

    nc.compile()
    return nc


def _pair_examples(nv):
    """Pair up examples to minimize the max pair sum (greedy fold + local
    search over pairwise re-pairings)."""
    nv = np.asarray(nv)
    order = list(np.argsort(-nv))
    n = len(order) // 2
    pairs = [[order[i], order[2 * n - 1 - i]] for i in range(n)]

    def ps(p):
        return int(nv[p[0]] + nv[p[1]])

    changed = True
    it = 0
    while changed and it < 1000:
        changed = False
        it += 1
        for i in range(n):
            for j in range(i + 1, n):
                a, b = pairs[i], pairs[j]
                cur = max(ps(a), ps(b))
                for (x, y) in (((a[0], b[0]), (a[1], b[1])),
                               ((a[0], b[1]), (a[1], b[0]))):
                    m = max(int(nv[x[0]] + nv[x[1]]), int(nv[y[0]] + nv[y[1]]))
                    if m < cur:
                        pairs[i], pairs[j] = list(x), list(y)
                        a, b = pairs[i], pairs[j]
                        cur = m
                        changed = True
    return pairs


def _expansion_patterns():
    """[16*128, 128] bf16: rows (side*8 + b)*128 + k, cols f=(gg,i,j).
    side 0 (subj): k = 16b + 8gg + i;  side 1 (obj): k = 16b + 8gg + j."""
    import ml_dtypes
    pat = np.zeros((2, 8, 128, 128), np.float32)
    gg, ii, jj = np.meshgrid(np.arange(2), np.arange(8), np.arange(8),
                             indexing="ij")
    f = (gg * 64 + ii * 8 + jj).ravel()
    for b in range(8):
        pat[0, b, (16 * b + gg * 8 + ii).ravel(), f] = 1.0
        pat[1, b, (16 * b + gg * 8 + jj).ravel(), f] = 1.0
    return pat.reshape(16 * 128, 128).astype(ml_dtypes.bfloat16)


# layout metadata shared between host_prep / assemble / kernel
_LAYOUT = None      # per pair: (bs, ts) arrays for packed columns
_SPILL = None       # list of (b, t) computed on host


def host_prep(hidden_states, entity_subw_indices, entity_subw_mask,
              triplet_entity_nums, pair_mask, W_fc):
    """Build per-core input maps (numpy only, cheap)."""
    global _LAST_C, _LAST_K, _LAYOUT, _SPILL
    import ml_dtypes
    bf16 = ml_dtypes.bfloat16
    hs = np.asarray(hidden_states, dtype=np.float32)
    idx = np.asarray(entity_subw_indices)
    msk = np.asarray(entity_subw_mask).astype(np.float32)
    trip = np.asarray(triplet_entity_nums)
    pm = np.asarray(pair_mask)
    # shuffle W to the SBUF chunk layout [p, (c, n)]
    w = (np.asarray(W_fc, dtype=np.float32).reshape(FC, 128, NT)
         .transpose(1, 0, 2).reshape(128, FC * NT).astype(bf16))

    cnt = np.maximum(msk.sum(axis=2), 1.0)          # (B, E)
    wgt = msk / cnt[:, :, None]                     # (B, E, S)

    # distinct hidden rows per example
    used = [np.unique(idx[b][msk[b] > 0]) for b in range(B)]
    K = max(1, int(np.ceil(max(len(u) for u in used) / 128)))
    KR = K * 128
    hidg = np.zeros((B, KR, H), bf16)
    at2 = np.zeros((B, KR, E), np.float32)
    for b in range(B):
        u = used[b]
        hidg[b, : len(u)] = hs[b, u].astype(bf16)
        pos = np.full(L, -1, np.int64)
        pos[u] = np.arange(len(u))
        e_i, s_i = np.nonzero(msk[b] > 0)
        np.add.at(at2[b], (pos[idx[b, e_i, s_i]], e_i), wgt[b, e_i, s_i])
    at2 = at2.astype(bf16)

    nv = pm.sum(axis=1).astype(np.int64)
    pairs = _pair_examples(nv)
    maxsum = max(int(nv[a] + nv[b]) for a, b in pairs)
    C = min(128, int(np.ceil(max(maxsum, 4) / 4) * 4))
    _LAST_C, _LAST_K = C, K

    KK = 2 * K
    gs = np.zeros((len(pairs), 128, C), bf16)
    go = np.zeros((len(pairs), 128, C), bf16)
    at_pair = np.zeros((len(pairs), KK * 128, 2 * E), bf16)
    hid_pair = np.zeros((len(pairs), KK * 128, H), bf16)
    colmap = []  # per pair: (bs array, ts array)
    spill = []
    for p, (bx, by) in enumerate(pairs):
        hid_pair[p, :KR] = hidg[bx]
        hid_pair[p, KR:] = hidg[by]
        at_pair[p, :KR, :E] = at2[bx]
        at_pair[p, KR:, E:] = at2[by]
        bs, ts = [], []
        k = 0
        for exl, b in ((0, bx), (1, by)):
            tv = np.nonzero(pm[b])[0]
            keep = min(len(tv), C - k)
            for t in tv[keep:]:
                spill.append((b, int(t)))
            tv = tv[:keep]
            n = len(tv)
            gs[p, exl * E + trip[b, tv, 0], k + np.arange(n)] = 1.0
            go[p, exl * E + trip[b, tv, 1], k + np.arange(n)] = 1.0
            bs.append(np.full(n, b))
            ts.append(tv)
            k += n
        colmap.append((np.concatenate(bs), np.concatenate(ts)))
    _LAYOUT = colmap
    _SPILL = [(b, t,
               _host_logits_row(hs, idx, wgt, trip, b, t, W_fc))
              for b, t in spill]

    pat = _expansion_patterns()
    in_maps = []
    for c in range(NCORES):
        p0 = c * NPAIR
        in_maps.append({
            "hid": np.ascontiguousarray(
                hid_pair[p0 : p0 + NPAIR].reshape(NPAIR * KK * 128, H)),
            "at": np.ascontiguousarray(
                at_pair[p0 : p0 + NPAIR].reshape(NPAIR * KK * 128, 2 * E)),
            "gs": np.ascontiguousarray(gs[p0 : p0 + NPAIR]),
            "go": np.ascontiguousarray(go[p0 : p0 + NPAIR]),
            "pat": pat,
            "w": w,
        })
    return in_maps


def _host_logits_row(hs, idx, wgt, trip, b, t, W_fc):
    """Exact logits (without bias) for one (b, t) triplet, in f64->f32."""
    e1, e2 = int(trip[b, t, 0]), int(trip[b, t, 1])
    subj = (wgt[b, e1][:, None] * hs[b, idx[b, e1]]).sum(0)
    obj = (wgt[b, e2][:, None] * hs[b, idx[b, e2]]).sum(0)
    bl = (subj.reshape(96, 8, 1) * obj.reshape(96, 1, 8)).reshape(-1)
    return bl @ np.asarray(W_fc, np.float32)


def assemble(results, b_fc):
    """results[c]["out"] is (NT, NPAIR, C) -> (B, T, NT) + bias."""
    bfc = np.asarray(b_fc, np.float32)
    logits = np.broadcast_to(bfc, (B, T, NT)).copy()
    for c in range(NCORES):
        o = np.asarray(results[c]["out"], np.float32)
        for P in range(NPAIR):
            bs, ts = _LAYOUT[c * NPAIR + P]
            n = len(bs)
            logits[bs, ts, :] = o[:, P, :n].T + bfc
    for b, t, row in _SPILL:
        logits[b, t, :] = row + bfc
    return logits


_NC_CACHE = {}


def kernel(hidden_states, entity_subw_indices, entity_subw_mask,
           triplet_entity_nums, pair_mask, W_fc, b_fc):
    in_maps = host_prep(hidden_states, entity_subw_indices, entity_subw_mask,
                        triplet_entity_nums, pair_mask, W_fc)
    key = (_LAST_C, _LAST_K)
    if key not in _NC_CACHE:
        _NC_CACHE[key] = build_program()
    nc = _NC_CACHE[key]
    res = run_bass_kernel_spmd(nc, in_maps, core_ids=list(range(NCORES)))
    return assemble(res.results, b_fc)


# revision 6
# speedup vs baseline: 2.7465x; 1.0465x over previous
"""Trainium2 Bass kernel for BilinearClassification (segment_reduce).

Math (per example b):
  ent[e,:]  = masked-mean over subword span of hidden[idx[e,s],:]      (E=64, H=768)
  subj[t,:] = ent[trip[t,0],:] * pm[t];  obj[t,:] = ent[trip[t,1],:] * pm[t]
  bl[t, (g,i,j)] = subj[t, g*8+i] * obj[t, g*8+j]                      (f = 6144)
  logits[t,n] = bl[t,:] @ W[:,n] + b[n]                                (NT=42)

Device strategy (8 cores, 4 examples = 2 example-pairs each, no collectives):
  - Only VALID triplets (pair_mask) are computed, packed into C<=128 columns
    per example-pair (host pairs examples to balance; the few overflow
    triplets are computed exactly on the host). Masked slots get b_fc.
  - Host gathers only the DISTINCT hidden rows each example references
    (K*128 rows instead of L=512), shrinking hid DMA and stage-1 PE.
  - stage 1: ent[(ex,e), h] = AT.T @ hidg per pair (2K contraction chunks).
  - stage 2 per pair, TRIPLET-MAJOR (t on partitions):
      subjT[t, h] = G_s.T @ ent   (2 matmuls), evac SBUF bf16; objT likewise.
      bl_t[t, (g,i,j)] = subjT[t,(g,i)] * objT[t,(g,j)]  -- broadcast-AP
          elementwise muls, split across DVE and GPSIMD (no PE, no PSUM).
      per 8-chunk group: PE-transpose bl_t chunks to [f, t] (PSUM bf16),
          evac (ACT/DVE alternating), logits += W_c.T @ blT_c.
  - host scatters packed columns back to (b, t) and adds b_fc.
"""
import sys

sys.path.insert(0, "/opt/trn_rl_repo")

import numpy as np

import concourse.bass as bass
import concourse.bacc as bacc
import concourse.tile as tile
from concourse import mybir
from concourse.bass_utils import run_bass_kernel_spmd

F32 = mybir.dt.float32
BF16 = mybir.dt.bfloat16

B, L, H = 32, 512, 768
E, S, T = 64, 8, 128
NT = 42
NCORES = 8
EXPC = B // NCORES          # 4 examples per core
NPAIR = EXPC // 2           # 2 example-pairs per core
FC = (H * 8) // 128         # 48 f-chunks
HH = H // 2

# data-dependent compile parameters (set by host_prep; defaults match the
# bundled fixed-seed inputs)
_LAST_C = 128               # packed triplet columns per pair (<= 128)
_LAST_K = 2                 # 128-row contraction chunks per example

GO = 8                      # f-chunks per transpose/FC group
MUL_CH = 4                  # f-chunks per elementwise-mul instruction
DVE_MULS = {0, 2, 4, 6, 8, 10, 5}   # mul-instr indices (of 12/pair) on DVE


def build_program(reps=1, C=None, K=None):
    C = _LAST_C if C is None else C
    K = _LAST_K if K is None else K
    nc = bacc.Bacc("TRN2", target_bir_lowering=False, debug=False)

    KK = 2 * K              # contraction chunks per pair
    hid_d = nc.dram_tensor("hid", (NPAIR * KK * 128, H), BF16, kind="ExternalInput")
    at_d = nc.dram_tensor("at", (NPAIR * KK * 128, 2 * E), BF16, kind="ExternalInput")
    gs_d = nc.dram_tensor("gs", (NPAIR, 128, C), BF16, kind="ExternalInput")
    go_d = nc.dram_tensor("go", (NPAIR, 128, C), BF16, kind="ExternalInput")
    id_d = nc.dram_tensor("ident", (C, C), BF16, kind="ExternalInput")
    # W pre-shuffled on host to the SBUF chunk layout [p, (c n)]
    w_d = nc.dram_tensor("w", (128, FC * NT), BF16, kind="ExternalInput")
    out_d = nc.dram_tensor("out", (NT, NPAIR, C), F32, kind="ExternalOutput")

    with tile.TileContext(nc) as tc:
        with (
            tc.tile_pool(name="consts", bufs=1) as consts,
            tc.tile_pool(name="hidp", bufs=2 * KK) as hidp,
            tc.tile_pool(name="atp", bufs=2) as atp,
            tc.tile_pool(name="entsb", bufs=1) as entsb,
            tc.tile_pool(name="gps", bufs=2, space="PSUM") as gps,
            tc.tile_pool(name="trps", bufs=2, space="PSUM") as trps,
            tc.tile_pool(name="lgps", bufs=1, space="PSUM") as lgps,
            tc.tile_pool(name="tsb", bufs=1) as tsb,
            tc.tile_pool(name="blp", bufs=1) as blp,
            tc.tile_pool(name="blsb", bufs=2) as blsb,
            tc.tile_pool(name="outp", bufs=1) as outp,
        ):
          for _rep in range(reps):
            # ---- input DMAs (pair 0's stage-1 inputs first)
            hid_t = [None] * NPAIR   # list of KK tiles [128, H] per pair
            at_t = [None] * NPAIR    # [128, KK, 2E]
            gs_t = [None] * NPAIR
            go_t = [None] * NPAIR
            hid_r = hid_d[:].rearrange("(pk p) h -> p pk h", p=128)
            at_r = at_d[:].rearrange("(pk p) e -> p pk e", p=128)
            for P in range(NPAIR):
                att = atp.tile([128, KK, 2 * E], BF16)
                nc.sync.dma_start(att[:], at_r[:, P * KK : (P + 1) * KK, :])
                at_t[P] = att
                hts = []
                for ck in range(KK):
                    h1 = hidp.tile([128, H], BF16)
                    nc.sync.dma_start(h1[:], hid_r[:, P * KK + ck, :])
                    hts.append(h1)
                hid_t[P] = hts
                g1 = consts.tile([128, C], BF16, tag=f"gs{P}")
                nc.sync.dma_start(g1[:], gs_d[P])
                gs_t[P] = g1
                g2 = consts.tile([128, C], BF16, tag=f"go{P}")
                nc.sync.dma_start(g2[:], go_d[P])
                go_t[P] = g2
            id_t = consts.tile([C, C], BF16, tag="ident")
            nc.sync.dma_start(id_t[:], id_d[:])
            w_all = consts.tile([128, FC, NT], BF16, tag="w")
            nc.sync.dma_start(
                w_all[:], w_d[:].rearrange("p (c n) -> p c n", n=NT))

            # ---- stage 1 for both pairs first (ent PSUM borrows a gps tile)
            ent_sb = []
            for P in range(NPAIR):
                esb = entsb.tile([128, H], BF16, tag=f"ent{P}")
                for nh in range(2):
                    ent_tile = gps.tile([128, 512], F32, tag="g")
                    ent_ps = ent_tile[:][:, :HH]
                    for ck in range(KK):
                        nc.tensor.matmul(
                            ent_ps,
                            at_t[P][:, ck, :],
                            hid_t[P][ck][:, nh * HH : (nh + 1) * HH],
                            start=(ck == 0),
                            stop=(ck == KK - 1),
                        )
                    if nh == 0:
                        nc.scalar.copy(esb[:, :HH], ent_ps)
                    else:
                        nc.vector.tensor_copy(esb[:, HH:], ent_ps)
                ent_sb.append(esb)

            # ---- stage 2
            lg = lgps.tile([NT, NPAIR, C], F32)
            for P in range(NPAIR):
                esb = ent_sb[P]
                # t-major gathers: subjT/objT [C(t), H]
                st_sb = tsb.tile([128, H], BF16, tag=f"sT{P}")
                ot_sb = tsb.tile([128, H], BF16, tag=f"oT{P}")
                for side, gmat, dst in ((0, gs_t[P], st_sb), (1, go_t[P], ot_sb)):
                    for nh in range(2):
                        t_tile = gps.tile([128, 512], F32, tag="g")
                        t_ps = t_tile[:][:C, :HH]
                        nc.tensor.matmul(
                            t_ps, gmat[:],
                            esb[:, nh * HH : (nh + 1) * HH],
                            start=True, stop=True)
                        if side == 0:
                            nc.scalar.copy(dst[:C, nh * HH : (nh + 1) * HH], t_ps)
                        else:
                            nc.vector.tensor_copy(
                                dst[:C, nh * HH : (nh + 1) * HH], t_ps)

                # bl_t[t, (c, gg, i, j)] muls, split DVE / GPSIMD
                blt = blp.tile([128, FC, 128], BF16, tag=f"bl{P}")
                s_r = st_sb[:C].rearrange("p (g i) -> p g i", i=8)
                o_r = ot_sb[:C].rearrange("p (g j) -> p g j", j=8)
                for m in range(FC // MUL_CH):
                    c0, c1 = m * MUL_CH, (m + 1) * MUL_CH
                    g0, g1_ = 2 * c0, 2 * c1
                    s_op = (s_r[:, g0:g1_, :].unsqueeze(3)
                            .broadcast_to((C, g1_ - g0, 8, 8)))
                    o_op = (o_r[:, g0:g1_, :].unsqueeze(2)
                            .broadcast_to((C, g1_ - g0, 8, 8)))
                    dst = blt[:C, c0:c1, :].rearrange(
                        "p c (gg i j) -> p (c gg) i j", gg=2, i=8)
                    if m in DVE_MULS:
                        nc.vector.tensor_mul(dst, s_op, o_op)
                    else:
                        nc.gpsimd.tensor_mul(dst, s_op, o_op)

                # transpose + evac + FC per 8-chunk group
                for g in range(FC // GO):
                    trp = trps.tile([128, GO, C], BF16, tag="tr")
                    for j in range(GO):
                        c = g * GO + j
                        nc.tensor.transpose(
                            trp[:, j, :C], blt[:C, c, :], id_t[:])
                    blT = blsb.tile([128, GO, C], BF16)
                    if g % 2 == 0:
                        nc.scalar.copy(blT[:], trp[:])
                    else:
                        nc.vector.tensor_copy(blT[:], trp[:])
                    for j in range(GO):
                        c = g * GO + j
                        nc.tensor.matmul(
                            lg[:, P, :],
                            w_all[:, c, :],
                            blT[:, j, :],
                            start=(c == 0),
                            stop=(c == FC - 1),
                        )

            out_sb = outp.tile([NT, NPAIR, C], F32)
            nc.scalar.copy(out_sb[:], lg[:])
            nc.sync.dma_start(out_d[:], out_sb[:])

    nc.compile()
    return nc


def _pair_examples(nv):
    """Pair up examples to minimize the max pair sum (greedy fold + local
    search over pairwise re-pairings)."""
    nv = np.asarray(nv)
    order = list(np.argsort(-nv))
    n = len(order) // 2
    pairs = [[order[i], order[2 * n - 1 - i]] for i in range(n)]

    def ps(p):
        return int(nv[p[0]] + nv[p[1]])

    changed = True
    it = 0
    while changed and it < 1000:
        changed = False
        it += 1
        for i in range(n):
            for j in range(i + 1, n):
                a, b = pairs[i], pairs[j]
                cur = max(ps(a), ps(b))
                for (x, y) in (((a[0], b[0]), (a[1], b[1])),
                               ((a[0], b[1]), (a[1], b[0]))):
                    m = max(int(nv[x[0]] + nv[x[1]]), int(nv[y[0]] + nv[y[1]]))
                    if m < cur:
                        pairs[i], pairs[j] = list(x), list(y)
                        a, b = pairs[i], pairs[j]
                        cur = m
                        changed = True
    return pairs


# layout metadata shared between host_prep / assemble / kernel
_LAYOUT = None      # per pair: (bs, ts) arrays for packed columns
_SPILL = None       # list of (b, t, logits_row) computed on host


def host_prep(hidden_states, entity_subw_indices, entity_subw_mask,
              triplet_entity_nums, pair_mask, W_fc):
    """Build per-core input maps (numpy only, cheap)."""
    global _LAST_C, _LAST_K, _LAYOUT, _SPILL
    import ml_dtypes
    bf16 = ml_dtypes.bfloat16
    hs = np.asarray(hidden_states, dtype=np.float32)
    idx = np.asarray(entity_subw_indices)
    msk = np.asarray(entity_subw_mask).astype(np.float32)
    trip = np.asarray(triplet_entity_nums)
    pm = np.asarray(pair_mask)
    # shuffle W to the SBUF chunk layout [p, (c, n)]
    w = (np.asarray(W_fc, dtype=np.float32).reshape(FC, 128, NT)
         .transpose(1, 0, 2).reshape(128, FC * NT).astype(bf16))

    cnt = np.maximum(msk.sum(axis=2), 1.0)          # (B, E)
    wgt = msk / cnt[:, :, None]                     # (B, E, S)

    # distinct hidden rows per example
    used = [np.unique(idx[b][msk[b] > 0]) for b in range(B)]
    K = max(1, int(np.ceil(max(len(u) for u in used) / 128)))
    KR = K * 128
    hidg = np.zeros((B, KR, H), bf16)
    at2 = np.zeros((B, KR, E), np.float32)
    for b in range(B):
        u = used[b]
        hidg[b, : len(u)] = hs[b, u].astype(bf16)
        pos = np.full(L, -1, np.int64)
        pos[u] = np.arange(len(u))
        e_i, s_i = np.nonzero(msk[b] > 0)
        np.add.at(at2[b], (pos[idx[b, e_i, s_i]], e_i), wgt[b, e_i, s_i])
    at2 = at2.astype(bf16)

    nv = pm.sum(axis=1).astype(np.int64)
    pairs = _pair_examples(nv)
    maxsum = max(int(nv[a] + nv[b]) for a, b in pairs)
    C = min(128, int(np.ceil(max(maxsum, 4) / 4) * 4))
    _LAST_C, _LAST_K = C, K

    KK = 2 * K
    gs = np.zeros((len(pairs), 128, C), bf16)
    go = np.zeros((len(pairs), 128, C), bf16)
    at_pair = np.zeros((len(pairs), KK * 128, 2 * E), bf16)
    hid_pair = np.zeros((len(pairs), KK * 128, H), bf16)
    colmap = []  # per pair: (bs array, ts array)
    spill = []
    for p, (bx, by) in enumerate(pairs):
        hid_pair[p, :KR] = hidg[bx]
        hid_pair[p, KR:] = hidg[by]
        at_pair[p, :KR, :E] = at2[bx]
        at_pair[p, KR:, E:] = at2[by]
        bs, ts = [], []
        k = 0
        for exl, b in ((0, bx), (1, by)):
            tv = np.nonzero(pm[b])[0]
            keep = min(len(tv), C - k)
            for t in tv[keep:]:
                spill.append((b, int(t)))
            tv = tv[:keep]
            n = len(tv)
            gs[p, exl * E + trip[b, tv, 0], k + np.arange(n)] = 1.0
            go[p, exl * E + trip[b, tv, 1], k + np.arange(n)] = 1.0
            bs.append(np.full(n, b))
            ts.append(tv)
            k += n
        colmap.append((np.concatenate(bs), np.concatenate(ts)))
    _LAYOUT = colmap
    _SPILL = [(b, t,
               _host_logits_row(hs, idx, wgt, trip, b, t, W_fc))
              for b, t in spill]

    in_maps = []
    ident = np.eye(C, dtype=bf16)
    for c in range(NCORES):
        p0 = c * NPAIR
        in_maps.append({
            "hid": np.ascontiguousarray(
                hid_pair[p0 : p0 + NPAIR].reshape(NPAIR * KK * 128, H)),
            "at": np.ascontiguousarray(
                at_pair[p0 : p0 + NPAIR].reshape(NPAIR * KK * 128, 2 * E)),
            "gs": np.ascontiguousarray(gs[p0 : p0 + NPAIR]),
            "go": np.ascontiguousarray(go[p0 : p0 + NPAIR]),
            "ident": ident,
            "w": w,
        })
    return in_maps


def _host_logits_row(hs, idx, wgt, trip, b, t, W_fc):
    """Exact logits (without bias) for one (b, t) triplet."""
    e1, e2 = int(trip[b, t, 0]), int(trip[b, t, 1])
    subj = (wgt[b, e1][:, None] * hs[b, idx[b, e1]]).sum(0)
    obj = (wgt[b, e2][:, None] * hs[b, idx[b, e2]]).sum(0)
    bl = (subj.reshape(96, 8, 1) * obj.reshape(96, 1, 8)).reshape(-1)
    return bl @ np.asarray(W_fc, np.float32)


def assemble(results, b_fc):
    """results[c]["out"] is (NT, NPAIR, C) -> (B, T, NT) + bias."""
    bfc = np.asarray(b_fc, np.float32)
    logits = np.broadcast_to(bfc, (B, T, NT)).copy()
    for c in range(NCORES):
        o = np.asarray(results[c]["out"], np.float32)
        for P in range(NPAIR):
            bs, ts = _LAYOUT[c * NPAIR + P]
            n = len(bs)
            logits[bs, ts, :] = o[:, P, :n].T + bfc
    for b, t, row in _SPILL:
        logits[b, t, :] = row + bfc
    return logits


_NC_CACHE = {}


def kernel(hidden_states, entity_subw_indices, entity_subw_mask,
           triplet_entity_nums, pair_mask, W_fc, b_fc):
    in_maps = host_prep(hidden_states, entity_subw_indices, entity_subw_mask,
                        triplet_entity_nums, pair_mask, W_fc)
    key = (_LAST_C, _LAST_K)
    if key not in _NC_CACHE:
        _NC_CACHE[key] = build_program()
    nc = _NC_CACHE[key]
    res = run_bass_kernel_spmd(nc, in_maps, core_ids=list(range(NCORES)))
    return assemble(res.results, b_fc)


# revision 9
# speedup vs baseline: 2.7850x; 1.0140x over previous
"""Trainium2 Bass kernel for BilinearClassification (segment_reduce).

Math (per example b):
  ent[e,:]  = masked-mean over subword span of hidden[idx[e,s],:]      (E=64, H=768)
  subj[t,:] = ent[trip[t,0],:] * pm[t];  obj[t,:] = ent[trip[t,1],:] * pm[t]
  bl[t, (g,i,j)] = subj[t, g*8+i] * obj[t, g*8+j]                      (f = 6144)
  logits[t,n] = bl[t,:] @ W[:,n] + b[n]                                (NT=42)

Device strategy (8 cores, 4 examples = 2 example-pairs each, no collectives):
  - Only VALID triplets (pair_mask) are computed, packed into C<=128 columns
    per example-pair (host pairs examples to balance; the few overflow
    triplets are computed exactly on the host). Masked slots get b_fc.
  - Host gathers only the DISTINCT hidden rows each example references
    (K*128 rows instead of L=512), shrinking hid DMA and stage-1 PE.
  - stage 1: ent[(ex,e), h] = AT.T @ hidg per pair (2K contraction chunks).
  - stage 2 per pair, TRIPLET-MAJOR (t on partitions):
      subjT[t, h] = G_s.T @ ent   (2 matmuls), evac SBUF bf16; objT likewise.
      bl_t[t, (g,i,j)] = subjT[t,(g,i)] * objT[t,(g,j)]  -- broadcast-AP
          elementwise muls, split across DVE and GPSIMD (no PE, no PSUM).
      per 8-chunk group: PE-transpose bl_t chunks to [f, t] (PSUM bf16),
          evac (ACT/DVE alternating), logits += W_c.T @ blT_c.
  - host scatters packed columns back to (b, t) and adds b_fc.
"""
import sys

sys.path.insert(0, "/opt/trn_rl_repo")

import numpy as np

import concourse.bass as bass
import concourse.bacc as bacc
import concourse.tile as tile
from concourse import mybir
from concourse.bass_utils import run_bass_kernel_spmd

F32 = mybir.dt.float32
BF16 = mybir.dt.bfloat16

B, L, H = 32, 512, 768
E, S, T = 64, 8, 128
NT = 42
NCORES = 8
EXPC = B // NCORES          # 4 examples per core
NPAIR = EXPC // 2           # 2 example-pairs per core
FC = (H * 8) // 128         # 48 f-chunks
HH = H // 2

# data-dependent compile parameters (set by host_prep; defaults match the
# bundled fixed-seed inputs)
_LAST_C = 128               # packed triplet columns per pair (<= 128)
_LAST_K = 2                 # 128-row contraction chunks per example

GO = 8                      # f-chunks per transpose/FC group
MUL_CH = 4                  # f-chunks per elementwise-mul instruction
DVE_MULS = {1, 4, 7, 10}    # mul-instr indices (of 12/pair) on DVE


def build_program(reps=1, C=None, K=None):
    C = _LAST_C if C is None else C
    K = _LAST_K if K is None else K
    nc = bacc.Bacc("TRN2", target_bir_lowering=False, debug=False)

    KK = 2 * K              # contraction chunks per pair
    hid_d = nc.dram_tensor("hid", (NPAIR * KK * 128, H), BF16, kind="ExternalInput")
    at_d = nc.dram_tensor("at", (NPAIR * KK * 128, 2 * E), BF16, kind="ExternalInput")
    gs_d = nc.dram_tensor("gs", (NPAIR, 128, C), BF16, kind="ExternalInput")
    go_d = nc.dram_tensor("go", (NPAIR, 128, C), BF16, kind="ExternalInput")
    id_d = nc.dram_tensor("ident", (C, C), BF16, kind="ExternalInput")
    # W pre-shuffled on host to the SBUF chunk layout [p, (c n)]
    w_d = nc.dram_tensor("w", (128, FC * NT), BF16, kind="ExternalInput")
    out_d = nc.dram_tensor("out", (NT, NPAIR, C), F32, kind="ExternalOutput")

    with tile.TileContext(nc) as tc:
        with (
            tc.tile_pool(name="consts", bufs=1) as consts,
            tc.tile_pool(name="hidp", bufs=2 * KK) as hidp,
            tc.tile_pool(name="atp", bufs=2) as atp,
            tc.tile_pool(name="entsb", bufs=1) as entsb,
            tc.tile_pool(name="gps", bufs=2, space="PSUM") as gps,
            tc.tile_pool(name="trps", bufs=2, space="PSUM") as trps,
            tc.tile_pool(name="lgps", bufs=1, space="PSUM") as lgps,
            tc.tile_pool(name="tsb", bufs=1) as tsb,
            tc.tile_pool(name="blp", bufs=4) as blp,
            tc.tile_pool(name="blsb", bufs=2) as blsb,
            tc.tile_pool(name="outp", bufs=1) as outp,
        ):
          for _rep in range(reps):
            # ---- input DMAs (pair 0's stage-1 inputs first)
            hid_t = [None] * NPAIR   # list of KK tiles [128, H] per pair
            at_t = [None] * NPAIR    # [128, KK, 2E]
            gs_t = [None] * NPAIR
            go_t = [None] * NPAIR
            hid_r = hid_d[:].rearrange("(pk p) h -> p pk h", p=128)
            at_r = at_d[:].rearrange("(pk p) e -> p pk e", p=128)
            for P in range(NPAIR):
                att = atp.tile([128, KK, 2 * E], BF16)
                nc.sync.dma_start(att[:], at_r[:, P * KK : (P + 1) * KK, :])
                at_t[P] = att
                hts = []
                for ck in range(KK):
                    h1 = hidp.tile([128, H], BF16)
                    nc.sync.dma_start(h1[:], hid_r[:, P * KK + ck, :])
                    hts.append(h1)
                hid_t[P] = hts
                g1 = consts.tile([128, C], BF16, tag=f"gs{P}")
                nc.sync.dma_start(g1[:], gs_d[P])
                gs_t[P] = g1
                g2 = consts.tile([128, C], BF16, tag=f"go{P}")
                nc.sync.dma_start(g2[:], go_d[P])
                go_t[P] = g2
            id_t = consts.tile([C, C], BF16, tag="ident")
            nc.sync.dma_start(id_t[:], id_d[:])
            w_all = consts.tile([128, FC, NT], BF16, tag="w")
            nc.sync.dma_start(
                w_all[:], w_d[:].rearrange("p (c n) -> p c n", n=NT))

            # ---- stage 1 for both pairs first (ent PSUM borrows a gps tile)
            ent_sb = []
            for P in range(NPAIR):
                esb = entsb.tile([128, H], BF16, tag=f"ent{P}")
                for nh in range(2):
                    ent_tile = gps.tile([128, 512], F32, tag="g")
                    ent_ps = ent_tile[:][:, :HH]
                    for ck in range(KK):
                        nc.tensor.matmul(
                            ent_ps,
                            at_t[P][:, ck, :],
                            hid_t[P][ck][:, nh * HH : (nh + 1) * HH],
                            start=(ck == 0),
                            stop=(ck == KK - 1),
                        )
                    if nh == 0:
                        nc.scalar.copy(esb[:, :HH], ent_ps)
                    else:
                        nc.vector.tensor_copy(esb[:, HH:], ent_ps)
                ent_sb.append(esb)

            # ---- stage 2
            lg = lgps.tile([NT, NPAIR, C], F32)
            for P in range(NPAIR):
                esb = ent_sb[P]
                # t-major gathers: subjT/objT [C(t), H]
                st_sb = tsb.tile([128, H], BF16, tag=f"sT{P}")
                ot_sb = tsb.tile([128, H], BF16, tag=f"oT{P}")
                for side, gmat, dst in ((0, gs_t[P], st_sb), (1, go_t[P], ot_sb)):
                    for nh in range(2):
                        t_tile = gps.tile([128, 512], F32, tag="g")
                        t_ps = t_tile[:][:C, :HH]
                        nc.tensor.matmul(
                            t_ps, gmat[:],
                            esb[:, nh * HH : (nh + 1) * HH],
                            start=True, stop=True)
                        if side == 0:
                            nc.scalar.copy(dst[:C, nh * HH : (nh + 1) * HH], t_ps)
                        else:
                            nc.vector.tensor_copy(
                                dst[:C, nh * HH : (nh + 1) * HH], t_ps)

                # bl_t[t, (c, gg, i, j)] muls, split DVE / GPSIMD; one bl
                # tile per transpose group so deps stay group-granular
                s_r = st_sb[:C].rearrange("p (g i) -> p g i", i=8)
                o_r = ot_sb[:C].rearrange("p (g j) -> p g j", j=8)
                blts = []
                for g in range(FC // GO):
                    blt = blp.tile([128, GO, 128], BF16)
                    blts.append(blt)
                    for mh in range(GO // MUL_CH):
                        m = g * (GO // MUL_CH) + mh
                        c0 = g * GO + mh * MUL_CH
                        c1 = c0 + MUL_CH
                        g0, g1_ = 2 * c0, 2 * c1
                        s_op = (s_r[:, g0:g1_, :].unsqueeze(3)
                                .broadcast_to((C, g1_ - g0, 8, 8)))
                        o_op = (o_r[:, g0:g1_, :].unsqueeze(2)
                                .broadcast_to((C, g1_ - g0, 8, 8)))
                        dst = blt[:C, mh * MUL_CH : (mh + 1) * MUL_CH, :].rearrange(
                            "p c (gg i j) -> p (c gg) i j", gg=2, i=8)
                        if m in DVE_MULS:
                            nc.vector.tensor_mul(dst, s_op, o_op)
                        else:
                            nc.gpsimd.tensor_mul(dst, s_op, o_op)

                # transpose + evac + FC per 8-chunk group
                for g in range(FC // GO):
                    blt = blts[g]
                    trp = trps.tile([128, GO, C], BF16, tag="tr")
                    for j in range(GO):
                        c = g * GO + j
                        nc.tensor.transpose(
                            trp[:, j, :C], blt[:C, j, :], id_t[:])
                    blT = blsb.tile([128, GO, C], BF16)
                    if g % 3 == 2:
                        nc.vector.tensor_copy(blT[:], trp[:])
                    else:
                        nc.scalar.copy(blT[:], trp[:])
                    for j in range(GO):
                        c = g * GO + j
                        nc.tensor.matmul(
                            lg[:, P, :],
                            w_all[:, c, :],
                            blT[:, j, :],
                            start=(c == 0),
                            stop=(c == FC - 1),
                        )

            out_sb = outp.tile([NT, NPAIR, C], F32)
            nc.scalar.copy(out_sb[:], lg[:])
            nc.sync.dma_start(out_d[:], out_sb[:])

    nc.compile()
    return nc


def _pair_examples(nv):
    """Pair up examples to minimize the max pair sum (greedy fold + local
    search over pairwise re-pairings)."""
    nv = np.asarray(nv)
    order = list(np.argsort(-nv))
    n = len(order) // 2
    pairs = [[order[i], order[2 * n - 1 - i]] for i in range(n)]

    def ps(p):
        return int(nv[p[0]] + nv[p[1]])

    changed = True
    it = 0
    while changed and it < 1000:
        changed = False
        it += 1
        for i in range(n):
            for j in range(i + 1, n):
                a, b = pairs[i], pairs[j]
                cur = max(ps(a), ps(b))
                for (x, y) in (((a[0], b[0]), (a[1], b[1])),
                               ((a[0], b[1]), (a[1], b[0]))):
                    m = max(int(nv[x[0]] + nv[x[1]]), int(nv[y[0]] + nv[y[1]]))
                    if m < cur:
                        pairs[i], pairs[j] = list(x), list(y)
                        a, b = pairs[i], pairs[j]
                        cur = m
                        changed = True
    return pairs


# layout metadata shared between host_prep / assemble / kernel
_LAYOUT = None      # per pair: (bs, ts) arrays for packed columns
_SPILL = None       # list of (b, t, logits_row) computed on host


def host_prep(hidden_states, entity_subw_indices, entity_subw_mask,
              triplet_entity_nums, pair_mask, W_fc):
    """Build per-core input maps (numpy only, cheap)."""
    global _LAST_C, _LAST_K, _LAYOUT, _SPILL
    import ml_dtypes
    bf16 = ml_dtypes.bfloat16
    hs = np.asarray(hidden_states, dtype=np.float32)
    idx = np.asarray(entity_subw_indices)
    msk = np.asarray(entity_subw_mask).astype(np.float32)
    trip = np.asarray(triplet_entity_nums)
    pm = np.asarray(pair_mask)
    # shuffle W to the SBUF chunk layout [p, (c, n)]
    w = (np.asarray(W_fc, dtype=np.float32).reshape(FC, 128, NT)
         .transpose(1, 0, 2).reshape(128, FC * NT).astype(bf16))

    cnt = np.maximum(msk.sum(axis=2), 1.0)          # (B, E)
    wgt = msk / cnt[:, :, None]                     # (B, E, S)

    # distinct hidden rows per example
    used = [np.unique(idx[b][msk[b] > 0]) for b in range(B)]
    K = max(1, int(np.ceil(max(len(u) for u in used) / 128)))
    KR = K * 128
    hidg = np.zeros((B, KR, H), bf16)
    at2 = np.zeros((B, KR, E), np.float32)
    for b in range(B):
        u = used[b]
        hidg[b, : len(u)] = hs[b, u].astype(bf16)
        pos = np.full(L, -1, np.int64)
        pos[u] = np.arange(len(u))
        e_i, s_i = np.nonzero(msk[b] > 0)
        np.add.at(at2[b], (pos[idx[b, e_i, s_i]], e_i), wgt[b, e_i, s_i])
    at2 = at2.astype(bf16)

    nv = pm.sum(axis=1).astype(np.int64)
    pairs = _pair_examples(nv)
    maxsum = max(int(nv[a] + nv[b]) for a, b in pairs)
    C = min(128, int(np.ceil(max(maxsum, 4) / 4) * 4))
    _LAST_C, _LAST_K = C, K

    KK = 2 * K
    gs = np.zeros((len(pairs), 128, C), bf16)
    go = np.zeros((len(pairs), 128, C), bf16)
    at_pair = np.zeros((len(pairs), KK * 128, 2 * E), bf16)
    hid_pair = np.zeros((len(pairs), KK * 128, H), bf16)
    colmap = []  # per pair: (bs array, ts array)
    spill = []
    for p, (bx, by) in enumerate(pairs):
        hid_pair[p, :KR] = hidg[bx]
        hid_pair[p, KR:] = hidg[by]
        at_pair[p, :KR, :E] = at2[bx]
        at_pair[p, KR:, E:] = at2[by]
        bs, ts = [], []
        k = 0
        for exl, b in ((0, bx), (1, by)):
            tv = np.nonzero(pm[b])[0]
            keep = min(len(tv), C - k)
            for t in tv[keep:]:
                spill.append((b, int(t)))
            tv = tv[:keep]
            n = len(tv)
            gs[p, exl * E + trip[b, tv, 0], k + np.arange(n)] = 1.0
            go[p, exl * E + trip[b, tv, 1], k + np.arange(n)] = 1.0
            bs.append(np.full(n, b))
            ts.append(tv)
            k += n
        colmap.append((np.concatenate(bs), np.concatenate(ts)))
    _LAYOUT = colmap
    _SPILL = [(b, t,
               _host_logits_row(hs, idx, wgt, trip, b, t, W_fc))
              for b, t in spill]

    in_maps = []
    ident = np.eye(C, dtype=bf16)
    for c in range(NCORES):
        p0 = c * NPAIR
        in_maps.append({
            "hid": np.ascontiguousarray(
                hid_pair[p0 : p0 + NPAIR].reshape(NPAIR * KK * 128, H)),
            "at": np.ascontiguousarray(
                at_pair[p0 : p0 + NPAIR].reshape(NPAIR * KK * 128, 2 * E)),
            "gs": np.ascontiguousarray(gs[p0 : p0 + NPAIR]),
            "go": np.ascontiguousarray(go[p0 : p0 + NPAIR]),
            "ident": ident,
            "w": w,
        })
    return in_maps


def _host_logits_row(hs, idx, wgt, trip, b, t, W_fc):
    """Exact logits (without bias) for one (b, t) triplet."""
    e1, e2 = int(trip[b, t, 0]), int(trip[b, t, 1])
    subj = (wgt[b, e1][:, None] * hs[b, idx[b, e1]]).sum(0)
    obj = (wgt[b, e2][:, None] * hs[b, idx[b, e2]]).sum(0)
    bl = (subj.reshape(96, 8, 1) * obj.reshape(96, 1, 8)).reshape(-1)
    return bl @ np.asarray(W_fc, np.float32)


def assemble(results, b_fc):
    """results[c]["out"] is (NT, NPAIR, C) -> (B, T, NT) + bias."""
    bfc = np.asarray(b_fc, np.float32)
    logits = np.broadcast_to(bfc, (B, T, NT)).copy()
    for c in range(NCORES):
        o = np.asarray(results[c]["out"], np.float32)
        for P in range(NPAIR):
            bs, ts = _LAYOUT[c * NPAIR + P]
            n = len(bs)
            logits[bs, ts, :] = o[:, P, :n].T + bfc
    for b, t, row in _SPILL:
        logits[b, t, :] = row + bfc
    return logits


_NC_CACHE = {}


def kernel(hidden_states, entity_subw_indices, entity_subw_mask,
           triplet_entity_nums, pair_mask, W_fc, b_fc):
    in_maps = host_prep(hidden_states, entity_subw_indices, entity_subw_mask,
                        triplet_entity_nums, pair_mask, W_fc)
    key = (_LAST_C, _LAST_K)
    if key not in _NC_CACHE:
        _NC_CACHE[key] = build_program()
    nc = _NC_CACHE[key]
    res = run_bass_kernel_spmd(nc, in_maps, core_ids=list(range(NCORES)))
    return assemble(res.results, b_fc)


# revision 11
# speedup vs baseline: 3.1362x; 1.1261x over previous
"""Trainium2 Bass kernel for BilinearClassification (segment_reduce).

Math (per example b):
  ent[e,:]  = masked-mean over subword span of hidden[idx[e,s],:]      (E=64, H=768)
  subj[t,:] = ent[trip[t,0],:] * pm[t];  obj[t,:] = ent[trip[t,1],:] * pm[t]
  bl[t, (g,i,j)] = subj[t, g*8+i] * obj[t, g*8+j]                      (f = 6144)
  logits[t,n] = bl[t,:] @ W[:,n] + b[n]                                (NT=42)

Device strategy (8 cores, 4 examples = 2 example-pairs each, no collectives):
  - Only VALID triplets (pair_mask) are computed, packed into C<=128 columns
    per example-pair (host pairs examples to balance; the few overflow
    triplets are computed exactly on the host). Masked slots get b_fc.
  - Host gathers only the DISTINCT hidden rows each example references
    (K*128 rows instead of L=512), shrinking hid DMA and stage-1 PE.
  - stage 1: ent[(ex,e), h] = AT.T @ hidg per pair (2K contraction chunks).
  - stage 2 per pair, TRIPLET-MAJOR (t on partitions):
      subjT[t, h] = G_s.T @ ent   (2 matmuls), evac SBUF bf16; objT likewise.
      bl_t[t, (g,i,j)] = subjT[t,(g,i)] * objT[t,(g,j)]  -- broadcast-AP
          elementwise muls, split across DVE and GPSIMD (no PE, no PSUM).
      per 8-chunk group: PE-transpose bl_t chunks to [f, t] (PSUM bf16),
          evac (ACT/DVE alternating), logits += W_c.T @ blT_c.
  - host scatters packed columns back to (b, t) and adds b_fc.
"""
import sys

sys.path.insert(0, "/opt/trn_rl_repo")

import numpy as np

import concourse.bass as bass
import concourse.bacc as bacc
import concourse.tile as tile
from concourse import mybir
from concourse.bass_utils import run_bass_kernel_spmd

F32 = mybir.dt.float32
BF16 = mybir.dt.bfloat16

B, L, H = 32, 512, 768
E, S, T = 64, 8, 128
NT = 42
NCORES = 8
EXPC = B // NCORES          # 4 examples per core
NPAIR = EXPC // 2           # 2 example-pairs per core
FC = (H * 8) // 128         # 48 f-chunks
HH = H // 2

# data-dependent compile parameters (set by host_prep; defaults match the
# bundled fixed-seed inputs)
_LAST_C = 128               # packed triplet columns per pair (<= 128)
_LAST_K = 2                 # 128-row contraction chunks per example

GO = 8                      # f-chunks per transpose/FC group
MUL_CH = 4                  # f-chunks per elementwise-mul instruction
DVE_MULS = {1, 4, 7, 10}    # mul-instr indices (of 12/pair) on DVE


def build_program(reps=1, C=None, K=None):
    C = _LAST_C if C is None else C
    K = _LAST_K if K is None else K
    nc = bacc.Bacc("TRN2", target_bir_lowering=False, debug=False)

    KK = 2 * K              # contraction chunks per pair
    hid_d = nc.dram_tensor("hid", (NPAIR * KK * 128, H), BF16, kind="ExternalInput")
    at_d = nc.dram_tensor("at", (NPAIR * KK * 128, 2 * E), BF16, kind="ExternalInput")
    gs_d = nc.dram_tensor("gs", (NPAIR, 128, C), BF16, kind="ExternalInput")
    go_d = nc.dram_tensor("go", (NPAIR, 128, C), BF16, kind="ExternalInput")
    id_d = nc.dram_tensor("ident", (C, C), BF16, kind="ExternalInput")
    # W pre-shuffled on host to the SBUF chunk layout [p, (c n)]
    w_d = nc.dram_tensor("w", (128, FC * NT), BF16, kind="ExternalInput")
    out_d = nc.dram_tensor("out", (NT, NPAIR, C), F32, kind="ExternalOutput")

    with tile.TileContext(nc) as tc:
        with (
            tc.tile_pool(name="consts", bufs=1) as consts,
            tc.tile_pool(name="hidp", bufs=2 * KK) as hidp,
            tc.tile_pool(name="atp", bufs=2) as atp,
            tc.tile_pool(name="entsb", bufs=1) as entsb,
            tc.tile_pool(name="gps", bufs=3, space="PSUM") as gps,
            tc.tile_pool(name="trps", bufs=3, space="PSUM") as trps,
            tc.tile_pool(name="lgps", bufs=1, space="PSUM") as lgps,
            tc.tile_pool(name="tsb", bufs=1) as tsb,
            tc.tile_pool(name="blp", bufs=4) as blp,
            tc.tile_pool(name="blsb", bufs=2) as blsb,
            tc.tile_pool(name="outp", bufs=1) as outp,
        ):
          for _rep in range(reps):
            # ---- input DMAs (pair 0's stage-1 inputs first)
            hid_t = [None] * NPAIR   # list of KK tiles [128, H] per pair
            at_t = [None] * NPAIR    # [128, KK, 2E]
            gs_t = [None] * NPAIR
            go_t = [None] * NPAIR
            hid_r = hid_d[:].rearrange("(pk p) h -> p pk h", p=128)
            at_r = at_d[:].rearrange("(pk p) e -> p pk e", p=128)
            for P in range(NPAIR):
                att = atp.tile([128, KK, 2 * E], BF16)
                nc.sync.dma_start(att[:], at_r[:, P * KK : (P + 1) * KK, :])
                at_t[P] = att
                hts = []
                for ck in range(KK):
                    h1 = hidp.tile([128, H], BF16)
                    nc.sync.dma_start(h1[:], hid_r[:, P * KK + ck, :])
                    hts.append(h1)
                hid_t[P] = hts
                g1 = consts.tile([128, C], BF16, tag=f"gs{P}")
                nc.sync.dma_start(g1[:], gs_d[P])
                gs_t[P] = g1
                g2 = consts.tile([128, C], BF16, tag=f"go{P}")
                nc.sync.dma_start(g2[:], go_d[P])
                go_t[P] = g2
                if P == 0:
                    id_t = consts.tile([C, C], BF16, tag="ident")
                    nc.sync.dma_start(id_t[:], id_d[:])
                    w_all = consts.tile([128, FC, NT], BF16, tag="w")
                    nc.sync.dma_start(
                        w_all[:], w_d[:].rearrange("p (c n) -> p c n", n=NT))

            # ---- stage 1 for both pairs first (ent PSUM borrows a gps tile)
            ent_sb = []
            for P in range(NPAIR):
                esb = entsb.tile([128, H], BF16, tag=f"ent{P}")
                for nh in range(2):
                    ent_tile = gps.tile([128, 512], F32, tag="g")
                    ent_ps = ent_tile[:][:, :HH]
                    for ck in range(KK):
                        nc.tensor.matmul(
                            ent_ps,
                            at_t[P][:, ck, :],
                            hid_t[P][ck][:, nh * HH : (nh + 1) * HH],
                            start=(ck == 0),
                            stop=(ck == KK - 1),
                        )
                    if nh == 0:
                        nc.scalar.copy(esb[:, :HH], ent_ps)
                    else:
                        nc.vector.tensor_copy(esb[:, HH:], ent_ps)
                ent_sb.append(esb)

            # ---- stage 2
            lg = lgps.tile([NT, NPAIR, C], F32)
            for P in range(NPAIR):
                esb = ent_sb[P]
                # t-major gathers: subjT/objT [C(t), H]
                st_sb = tsb.tile([128, H], BF16, tag=f"sT{P}")
                ot_sb = tsb.tile([128, H], BF16, tag=f"oT{P}")
                for side, gmat, dst in ((0, gs_t[P], st_sb), (1, go_t[P], ot_sb)):
                    for nh in range(2):
                        t_tile = gps.tile([128, 512], F32, tag="g")
                        t_ps = t_tile[:][:C, :HH]
                        nc.tensor.matmul(
                            t_ps, gmat[:],
                            esb[:, nh * HH : (nh + 1) * HH],
                            start=True, stop=True)
                        if side == 0:
                            nc.scalar.copy(dst[:C, nh * HH : (nh + 1) * HH], t_ps)
                        else:
                            nc.vector.tensor_copy(
                                dst[:C, nh * HH : (nh + 1) * HH], t_ps)

                # bl_t[t, (c, gg, i, j)] muls, split DVE / GPSIMD; one bl
                # tile per transpose group so deps stay group-granular
                s_r = st_sb[:C].rearrange("p (g i) -> p g i", i=8)
                o_r = ot_sb[:C].rearrange("p (g j) -> p g j", j=8)
                blts = []
                for g in range(FC // GO):
                    blt = blp.tile([128, GO, 128], BF16)
                    blts.append(blt)
                    for mh in range(GO // MUL_CH):
                        m = g * (GO // MUL_CH) + mh
                        c0 = g * GO + mh * MUL_CH
                        c1 = c0 + MUL_CH
                        g0, g1_ = 2 * c0, 2 * c1
                        s_op = (s_r[:, g0:g1_, :].unsqueeze(3)
                                .broadcast_to((C, g1_ - g0, 8, 8)))
                        o_op = (o_r[:, g0:g1_, :].unsqueeze(2)
                                .broadcast_to((C, g1_ - g0, 8, 8)))
                        dst = blt[:C, mh * MUL_CH : (mh + 1) * MUL_CH, :].rearrange(
                            "p c (gg i j) -> p (c gg) i j", gg=2, i=8)
                        if m in DVE_MULS:
                            nc.vector.tensor_mul(dst, s_op, o_op)
                        else:
                            nc.gpsimd.tensor_mul(dst, s_op, o_op)

                # transpose + evac + FC per 8-chunk group
                for g in range(FC // GO):
                    blt = blts[g]
                    trp = trps.tile([128, GO, C], BF16, tag="tr")
                    for j in range(GO):
                        c = g * GO + j
                        nc.tensor.transpose(
                            trp[:, j, :C], blt[:C, j, :], id_t[:])
                    blT = blsb.tile([128, GO, C], BF16)
                    if g % 3 == 2:
                        nc.vector.tensor_copy(blT[:], trp[:])
                    else:
                        nc.scalar.copy(blT[:], trp[:])
                    for j in range(GO):
                        c = g * GO + j
                        nc.tensor.matmul(
                            lg[:, P, :],
                            w_all[:, c, :],
                            blT[:, j, :],
                            start=(c == 0),
                            stop=(c == FC - 1),
                        )

            out_sb = outp.tile([NT, NPAIR, C], F32)
            nc.scalar.copy(out_sb[:], lg[:])
            nc.sync.dma_start(out_d[:], out_sb[:])

    nc.compile()
    return nc


def _pair_examples(nv):
    """Pair up examples to minimize the max pair sum (greedy fold + local
    search over pairwise re-pairings)."""
    nv = np.asarray(nv)
    order = list(np.argsort(-nv))
    n = len(order) // 2
    pairs = [[order[i], order[2 * n - 1 - i]] for i in range(n)]

    def ps(p):
        return int(nv[p[0]] + nv[p[1]])

    changed = True
    it = 0
    while changed and it < 1000:
        changed = False
        it += 1
        for i in range(n):
            for j in range(i + 1, n):
                a, b = pairs[i], pairs[j]
                cur = max(ps(a), ps(b))
                for (x, y) in (((a[0], b[0]), (a[1], b[1])),
                               ((a[0], b[1]), (a[1], b[0]))):
                    m = max(int(nv[x[0]] + nv[x[1]]), int(nv[y[0]] + nv[y[1]]))
                    if m < cur:
                        pairs[i], pairs[j] = list(x), list(y)
                        a, b = pairs[i], pairs[j]
                        cur = m
                        changed = True
    return pairs


# layout metadata shared between host_prep / assemble / kernel
_LAYOUT = None      # per pair: (bs, ts) arrays for packed columns
_SPILL = None       # list of (b, t, logits_row) computed on host


def host_prep(hidden_states, entity_subw_indices, entity_subw_mask,
              triplet_entity_nums, pair_mask, W_fc):
    """Build per-core input maps (numpy only, cheap)."""
    global _LAST_C, _LAST_K, _LAYOUT, _SPILL
    import ml_dtypes
    bf16 = ml_dtypes.bfloat16
    hs = np.asarray(hidden_states, dtype=np.float32)
    idx = np.asarray(entity_subw_indices)
    msk = np.asarray(entity_subw_mask).astype(np.float32)
    trip = np.asarray(triplet_entity_nums)
    pm = np.asarray(pair_mask)
    # shuffle W to the SBUF chunk layout [p, (c, n)]
    w = (np.asarray(W_fc, dtype=np.float32).reshape(FC, 128, NT)
         .transpose(1, 0, 2).reshape(128, FC * NT).astype(bf16))

    cnt = np.maximum(msk.sum(axis=2), 1.0)          # (B, E)
    wgt = msk / cnt[:, :, None]                     # (B, E, S)

    # distinct hidden rows per example
    used = [np.unique(idx[b][msk[b] > 0]) for b in range(B)]
    K = max(1, int(np.ceil(max(len(u) for u in used) / 128)))
    KR = K * 128
    hidg = np.zeros((B, KR, H), bf16)
    at2 = np.zeros((B, KR, E), np.float32)
    for b in range(B):
        u = used[b]
        hidg[b, : len(u)] = hs[b, u].astype(bf16)
        pos = np.full(L, -1, np.int64)
        pos[u] = np.arange(len(u))
        e_i, s_i = np.nonzero(msk[b] > 0)
        np.add.at(at2[b], (pos[idx[b, e_i, s_i]], e_i), wgt[b, e_i, s_i])
    at2 = at2.astype(bf16)

    nv = pm.sum(axis=1).astype(np.int64)
    pairs = _pair_examples(nv)
    maxsum = max(int(nv[a] + nv[b]) for a, b in pairs)
    C = min(128, int(np.ceil(max(maxsum, 4) / 4) * 4))
    _LAST_C, _LAST_K = C, K

    KK = 2 * K
    gs = np.zeros((len(pairs), 128, C), bf16)
    go = np.zeros((len(pairs), 128, C), bf16)
    at_pair = np.zeros((len(pairs), KK * 128, 2 * E), bf16)
    hid_pair = np.zeros((len(pairs), KK * 128, H), bf16)
    colmap = []  # per pair: (bs array, ts array)
    spill = []
    for p, (bx, by) in enumerate(pairs):
        hid_pair[p, :KR] = hidg[bx]
        hid_pair[p, KR:] = hidg[by]
        at_pair[p, :KR, :E] = at2[bx]
        at_pair[p, KR:, E:] = at2[by]
        bs, ts = [], []
        k = 0
        for exl, b in ((0, bx), (1, by)):
            tv = np.nonzero(pm[b])[0]
            keep = min(len(tv), C - k)
            for t in tv[keep:]:
                spill.append((b, int(t)))
            tv = tv[:keep]
            n = len(tv)
            gs[p, exl * E + trip[b, tv, 0], k + np.arange(n)] = 1.0
            go[p, exl * E + trip[b, tv, 1], k + np.arange(n)] = 1.0
            bs.append(np.full(n, b))
            ts.append(tv)
            k += n
        colmap.append((np.concatenate(bs), np.concatenate(ts)))
    _LAYOUT = colmap
    _SPILL = [(b, t,
               _host_logits_row(hs, idx, wgt, trip, b, t, W_fc))
              for b, t in spill]

    in_maps = []
    ident = np.eye(C, dtype=bf16)
    for c in range(NCORES):
        p0 = c * NPAIR
        in_maps.append({
            "hid": np.ascontiguousarray(
                hid_pair[p0 : p0 + NPAIR].reshape(NPAIR * KK * 128, H)),
            "at": np.ascontiguousarray(
                at_pair[p0 : p0 + NPAIR].reshape(NPAIR * KK * 128, 2 * E)),
            "gs": np.ascontiguousarray(gs[p0 : p0 + NPAIR]),
            "go": np.ascontiguousarray(go[p0 : p0 + NPAIR]),
            "ident": ident,
            "w": w,
        })
    return in_maps


def _host_logits_row(hs, idx, wgt, trip, b, t, W_fc):
    """Exact logits (without bias) for one (b, t) triplet."""
    e1, e2 = int(trip[b, t, 0]), int(trip[b, t, 1])
    subj = (wgt[b, e1][:, None] * hs[b, idx[b, e1]]).sum(0)
    obj = (wgt[b, e2][:, None] * hs[b, idx[b, e2]]).sum(0)
    bl = (subj.reshape(96, 8, 1) * obj.reshape(96, 1, 8)).reshape(-1)
    return bl @ np.asarray(W_fc, np.float32)


def assemble(results, b_fc):
    """results[c]["out"] is (NT, NPAIR, C) -> (B, T, NT) + bias."""
    bfc = np.asarray(b_fc, np.float32)
    logits = np.broadcast_to(bfc, (B, T, NT)).copy()
    for c in range(NCORES):
        o = np.asarray(results[c]["out"], np.float32)
        for P in range(NPAIR):
            bs, ts = _LAYOUT[c * NPAIR + P]
            n = len(bs)
            logits[bs, ts, :] = o[:, P, :n].T + bfc
    for b, t, row in _SPILL:
        logits[b, t, :] = row + bfc
    return logits


_NC_CACHE = {}


def kernel(hidden_states, entity_subw_indices, entity_subw_mask,
           triplet_entity_nums, pair_mask, W_fc, b_fc):
    in_maps = host_prep(hidden_states, entity_subw_indices, entity_subw_mask,
                        triplet_entity_nums, pair_mask, W_fc)
    key = (_LAST_C, _LAST_K)
    if key not in _NC_CACHE:
        _NC_CACHE[key] = build_program()
    nc = _NC_CACHE[key]
    res = run_bass_kernel_spmd(nc, in_maps, core_ids=list(range(NCORES)))
    return assemble(res.results, b_fc)
